# revision 23
# baseline (speedup 1.0000x reference)
"""Trainium2 Bass kernel: Conv2d(1->64, k=7, valid) on data [32,1,224,224] f32.

Data-parallel over batch (4 images per core on 8 cores).  Per core:
im2col matmul in fp16 (fp32 PSUM).  Two measured hardware limits drive
the design: the 16 DMA engines sustain ~264 GB/s of aggregate READ
traffic (SBUF or HBM alike), and the PE duty-cycle throttle caps
sustained streaming at ~2.4 cols/ns (two concurrent 64-col row groups
at 1.2 GHz effective).

The kernel balances the two: 12 of 16 tile-pairs materialize only 4 of
the 7 kx shifts in the im2col (fewer DMA bytes, but 2 PSUM-accumulating
matmuls per chunk = 2x tensor time); 4 pairs materialize all 7
(full-rate single matmul, more DMA).  Outputs leave as fp16.

Layout/pipeline (per core, 32 row-block "tiles" of 28 output rows,
processed as 16 pairs):
  - host: builds the partial im2col per tile in DRAM: KP rows (7 ky x
    KXL kx shifts), each a contiguous run of 28*224+8 fp16 elements.
  - i2c: one contiguous [KP, RUN] DMA per tile from DRAM.  Pair
    layout: tile A at partitions 0.. (PE row group h0), B at 64.. (h1).
  - matmul: per 448-col PSUM chunk, NMAT matmuls per tile half, halves
    dual-issued on the PE; matmul m reads the rhs at free-dim offset
    m*KXL (zero weight rows pad kx=7).
  - copy: psum [128,448] f32 -> ob fp16, alternating DVE/ACT.
  - out: fp16 stores (no cast) on the sync HWDGE queue, one DMA per
    tile [64ch, nrows*224].  Cols 218..223 are garbage (kx wrap) and
    are sliced off on the host, which also does the lossless
    fp16->fp32 cast of the result.
"""

import numpy as np

B = 32            # full batch
OC = 64           # out channels
KS = 7            # kernel size
H = 224           # input H=W
OH = 218          # valid output rows/cols
OW = 224          # im2col row width (incl 6 garbage cols)
NCORES = 8
IPC = B // NCORES  # images per core

BLK = 28          # output rows per tile
NBLK = 8          # tiles per image (7x28 + 1x22 valid rows)
NTILES = IPC * NBLK
NPAIRS = NTILES // 2
NCOLS = BLK * OW  # 6272 im2col columns per tile
RUN = NCOLS + 8   # per-partition run (covers kx shifts)
CHUNK = 448       # psum chunk columns
NCHUNK = NCOLS // CHUNK  # 14
OBW = CHUNK // OW * OH   # 436 ob columns per chunk (garbage stripped)

KPMAX = KS * KS   # 49 im2col partitions for a full (KXL=7) tile

# per-pair kx materialization: 4 -> [28,RUN] i2c + 2 matmuls/chunk,
# 7 -> [49,RUN] i2c + 1 matmul/chunk.  12:4 mix balances the ~264 GB/s
# DMA-read limit against the ~2.4 col/ns sustained PE limit.
PAIR_KXL = [4, 4, 7, 4, 4, 7, 4, 4, 4, 7, 4, 4, 4, 7, 4, 7]

_CACHE = {}


def _build():
    import concourse.mybir as mybir
    import concourse.tile as tile
    from concourse import bacc

    nc = bacc.Bacc("TRN2", target_bir_lowering=False, debug=False)

    # [KP, NTILES, RUN] so one tile's packets stripe across DRAM banks
    i2cd = nc.dram_tensor("i2cd", [KPMAX, NTILES, RUN], mybir.dt.float16,
                          kind="ExternalInput")
    # wbd[g, :, 0:64] = KXL7 weights; [g, :, 64+64m : 128+64m] = KXL4 m-th
    wbd = nc.dram_tensor("wbd", [2, KPMAX, 3 * OC], mybir.dt.float16,
                         kind="ExternalInput")
    out = nc.dram_tensor("out", [IPC, OC, OH, OH], mybir.dt.float16,
                         kind="ExternalOutput")

    import concourse.bass as bass

    with tile.TileContext(nc) as tc:
        with (
            tc.tile_pool(name="wp", bufs=1) as w_pool,
            tc.tile_pool(name="i2c", bufs=6) as i2c_pool,
            tc.tile_pool(name="ob", bufs=4) as ob_pool,
            tc.tile_pool(name="ps", bufs=8, space="PSUM") as ps_pool,
        ):
            wt = w_pool.tile([128, 3 * OC], mybir.dt.float16)
            nc.gpsimd.dma_start(out=wt[0:KPMAX, :], in_=wbd[0, :, :])
            nc.gpsimd.dma_start(out=wt[64:64 + KPMAX, :], in_=wbd[1, :, :])

            PF = 5
            i2c_tiles = {}

            def issue_i2c(q):
                kp = KS * PAIR_KXL[q]
                i2c = i2c_pool.tile([128, RUN], mybir.dt.float16,
                                    tag="i2c", name=f"i2c{q}")
                for half in range(2):
                    nc.gpsimd.dma_start(
                        out=i2c[64 * half:64 * half + kp, :],
                        in_=i2cd[0:kp, 2 * q + half, :])
                i2c_tiles[q] = i2c

            for q in range(min(PF, NPAIRS)):
                issue_i2c(q)

            for q in range(NPAIRS):
                kxl = PAIR_KXL[q]
                nmat = -(-KS // kxl)
                kp = KS * kxl

                if q + PF < NPAIRS:
                    issue_i2c(q + PF)
                i2c = i2c_tiles.pop(q)

                ob = ob_pool.tile([128, NCHUNK * OBW], mybir.dt.float16,
                                  tag="ob")
                for j in range(NCHUNK):
                    ps = ps_pool.tile([128, CHUNK], mybir.dt.float32,
                                      tag="ps")
                    c0 = CHUNK * j
                    for m in range(nmat):
                        st, sp = (m == 0), (m == nmat - 1)
                        wc = 0 if kxl == KS else OC * (1 + m)
                        nc.tensor.matmul(
                            ps[0:OC, :], wt[0:kp, wc:wc + OC],
                            i2c[0:kp, c0 + m * kxl:c0 + m * kxl + CHUNK],
                            start=st, stop=sp)
                        nc.tensor.matmul(
                            ps[OC:128, :], wt[64:64 + kp, wc:wc + OC],
                            i2c[64:64 + kp, c0 + m * kxl:c0 + m * kxl + CHUNK],
                            start=st, stop=sp)
                    # strip the 6 garbage cols of each 224-col output row
                    pssrc = bass.AP(
                        tensor=ps.tensor, offset=ps.offset,
                        ap=[[ps.ap[0][0], 128], [OW, CHUNK // OW], [1, OH]],
                    )
                    if j % 2 == 0:
                        nc.vector.tensor_copy(
                            ob[:, OBW * j:OBW * (j + 1)], pssrc)
                    else:
                        nc.scalar.copy(ob[:, OBW * j:OBW * (j + 1)], pssrc)

                for half in range(2):
                    t = 2 * q + half
                    imgi, blk = divmod(t, NBLK)
                    r0 = BLK * blk
                    nrows = min(BLK, OH - r0)
                    nc.gpsimd.dma_start(
                        out=out[imgi, :, r0:r0 + nrows, :],
                        in_=ob[64 * half:64 * half + OC, :nrows * OH])

    nc.compile()
    return nc


def _prep_inputs(data, weight):
    d = np.asarray(data).reshape(B, H, H).astype(np.float16)
    dpad = np.zeros((B, 256, H), dtype=np.float16)
    dpad[:, :H, :] = d
    dflat = dpad.reshape(B, 256 * H)
    w = np.asarray(weight).reshape(OC, KS, KS).astype(np.float16)

    wbd = np.zeros((2, KPMAX, 3 * OC), dtype=np.float16)
    wbd[:, :, 0:OC] = w.reshape(OC, KS * KS).T
    for m in range(2):
        for ky in range(KS):
            for kxl in range(4):
                kx = m * 4 + kxl
                if kx >= KS:
                    continue
                wbd[:, ky * 4 + kxl, OC * (1 + m):OC * (2 + m)] = w[:, ky, kx]

    in_maps = []
    for c in range(NCORES):
        i2cd = np.zeros((KPMAX, NTILES, RUN), dtype=np.float16)
        for t in range(NTILES):
            imgi, blk = divmod(t, NBLK)
            g = c * IPC + imgi
            r0 = BLK * blk
            kxl = PAIR_KXL[t // 2]
            for ky in range(KS):
                base = (r0 + ky) * H
                for kxi in range(kxl):
                    i2cd[ky * kxl + kxi, t, :] = \
                        dflat[g, base + kxi:base + kxi + RUN]
        in_maps.append({"i2cd": i2cd, "wbd": wbd})
    return in_maps


def kernel(data, weight):
    from concourse.bass_utils import run_bass_kernel_spmd

    if "nc" not in _CACHE:
        _CACHE["nc"] = _build()
    nc = _CACHE["nc"]

    in_maps = _prep_inputs(np.asarray(data), np.asarray(weight))
    res = run_bass_kernel_spmd(nc, in_maps, core_ids=list(range(NCORES)))
    outs = [r["out"] for r in res.results]
    full = np.concatenate(outs, axis=0)  # [32, 64, 218, 218] f16
    return full.astype(np.float32)


# revision 25
# speedup vs baseline: 1.0137x; 1.0137x over previous
"""Trainium2 Bass kernel: Conv2d(1->64, k=7, valid) on data [32,1,224,224] f32.

Data-parallel over batch (4 images per core on 8 cores).  Per core:
im2col matmul in fp16 (fp32 PSUM).  Two measured hardware limits drive
the design: the 16 DMA engines sustain ~264 GB/s of aggregate READ
traffic (SBUF or HBM alike), and the PE duty-cycle throttle caps
sustained streaming at ~2.4 cols/ns (two concurrent 64-col row groups
at 1.2 GHz effective).

The kernel balances the two: 12 of 16 tile-pairs materialize only 4 of
the 7 kx shifts in the im2col (fewer DMA bytes, but 2 PSUM-accumulating
matmuls per chunk = 2x tensor time); 4 pairs materialize all 7
(full-rate single matmul, more DMA).  Outputs leave as fp16.

Layout/pipeline (per core, 32 row-block "tiles" of 28 output rows,
processed as 16 pairs):
  - host: builds the partial im2col per tile in DRAM: KP rows (7 ky x
    KXL kx shifts), each a contiguous run of 28*224+8 fp16 elements.
  - i2c: one contiguous [KP, RUN] DMA per tile from DRAM.  Pair
    layout: tile A at partitions 0.. (PE row group h0), B at 64.. (h1).
  - matmul: per 448-col PSUM chunk, NMAT matmuls per tile half, halves
    dual-issued on the PE; matmul m reads the rhs at free-dim offset
    m*KXL (zero weight rows pad kx=7).
  - copy: psum [128,448] f32 -> ob fp16, alternating DVE/ACT.
  - out: fp16 stores (no cast) on the sync HWDGE queue, one DMA per
    tile [64ch, nrows*224].  Cols 218..223 are garbage (kx wrap) and
    are sliced off on the host, which also does the lossless
    fp16->fp32 cast of the result.
"""

import numpy as np

B = 32            # full batch
OC = 64           # out channels
KS = 7            # kernel size
H = 224           # input H=W
OH = 218          # valid output rows/cols
OW = 224          # im2col row width (incl 6 garbage cols)
NCORES = 8
IPC = B // NCORES  # images per core

BLK = 28          # output rows per tile
NBLK = 8          # tiles per image (7x28 + 1x22 valid rows)
NTILES = IPC * NBLK
NPAIRS = NTILES // 2
NCOLS = BLK * OW  # 6272 im2col columns per tile
RUN = NCOLS + 8   # per-partition run (covers kx shifts)
CHUNK = 448       # psum chunk columns
NCHUNK = NCOLS // CHUNK  # 14
OBW = CHUNK // OW * OH   # 436 ob columns per chunk (garbage stripped)

KPMAX = KS * KS   # 49 im2col partitions for a full (KXL=7) tile

# per-pair kx materialization: 4 -> [28,RUN] i2c + 2 matmuls/chunk,
# 7 -> [49,RUN] i2c + 1 matmul/chunk.  12:4 mix balances the ~264 GB/s
# DMA-read limit against the ~2.4 col/ns sustained PE limit.
PAIR_KXL = [4, 4, 7, 4, 4, 7, 4, 4, 4, 7, 4, 4, 4, 7, 4, 7]

_CACHE = {}


def _build():
    import concourse.mybir as mybir
    import concourse.tile as tile
    from concourse import bacc

    nc = bacc.Bacc("TRN2", target_bir_lowering=False, debug=False)

    # [KP, NTILES, RUN] so one tile's packets stripe across DRAM banks
    i2cd = nc.dram_tensor("i2cd", [KPMAX, NTILES, RUN], mybir.dt.float16,
                          kind="ExternalInput")
    # wbd[g, :, 0:64] = KXL7 weights; [g, :, 64+64m : 128+64m] = KXL4 m-th
    wbd = nc.dram_tensor("wbd", [2, KPMAX, 3 * OC], mybir.dt.float16,
                         kind="ExternalInput")
    out = nc.dram_tensor("out", [IPC, OC, OH, OH], mybir.dt.float16,
                         kind="ExternalOutput")

    import concourse.bass as bass

    with tile.TileContext(nc) as tc:
        with (
            tc.tile_pool(name="wp", bufs=1) as w_pool,
            tc.tile_pool(name="i2c", bufs=6) as i2c_pool,
            tc.tile_pool(name="ob", bufs=4) as ob_pool,
            tc.tile_pool(name="ps", bufs=8, space="PSUM") as ps_pool,
        ):
            wt = w_pool.tile([128, 3 * OC], mybir.dt.float16)
            nc.gpsimd.dma_start(out=wt[0:KPMAX, :], in_=wbd[0, :, :])
            nc.gpsimd.dma_start(out=wt[64:64 + KPMAX, :], in_=wbd[1, :, :])

            PF = 5
            i2c_tiles = {}

            def issue_i2c(q):
                kp = KS * PAIR_KXL[q]
                i2c = i2c_pool.tile([128, RUN], mybir.dt.float16,
                                    tag="i2c", name=f"i2c{q}")
                for half in range(2):
                    nc.gpsimd.dma_start(
                        out=i2c[64 * half:64 * half + kp, :],
                        in_=i2cd[0:kp, 2 * q + half, :])
                i2c_tiles[q] = i2c

            for q in range(min(PF, NPAIRS)):
                issue_i2c(q)

            for q in range(NPAIRS):
                kxl = PAIR_KXL[q]
                nmat = -(-KS // kxl)
                kp = KS * kxl

                if q + PF < NPAIRS:
                    issue_i2c(q + PF)
                i2c = i2c_tiles.pop(q)

                ob = ob_pool.tile([128, NCHUNK * OBW], mybir.dt.float16,
                                  tag="ob")
                for j in range(NCHUNK):
                    ps = ps_pool.tile([128, CHUNK], mybir.dt.float32,
                                      tag="ps")
                    c0 = CHUNK * j
                    for m in range(nmat):
                        st, sp = (m == 0), (m == nmat - 1)
                        wc = 0 if kxl == KS else OC * (1 + m)
                        nc.tensor.matmul(
                            ps[0:OC, :], wt[0:kp, wc:wc + OC],
                            i2c[0:kp, c0 + m * kxl:c0 + m * kxl + CHUNK],
                            start=st, stop=sp)
                        nc.tensor.matmul(
                            ps[OC:128, :], wt[64:64 + kp, wc:wc + OC],
                            i2c[64:64 + kp, c0 + m * kxl:c0 + m * kxl + CHUNK],
                            start=st, stop=sp)
                    # strip the 6 garbage cols of each 224-col output row
                    pssrc = bass.AP(
                        tensor=ps.tensor, offset=ps.offset,
                        ap=[[ps.ap[0][0], 128], [OW, CHUNK // OW], [1, OH]],
                    )
                    if j % 2 == 0:
                        nc.vector.tensor_copy(
                            ob[:, OBW * j:OBW * (j + 1)], pssrc)
                    else:
                        nc.scalar.copy(ob[:, OBW * j:OBW * (j + 1)], pssrc)

                for half in range(2):
                    t = 2 * q + half
                    imgi, blk = divmod(t, NBLK)
                    r0 = BLK * blk
                    nrows = min(BLK, OH - r0)
                    nc.gpsimd.dma_start(
                        out=out[imgi, :, r0:r0 + nrows, :],
                        in_=ob[64 * half:64 * half + OC, :nrows * OH])

    nc.compile()
    return nc


def _prep_inputs(data, weight):
    d = np.asarray(data).reshape(B, H, H).astype(np.float16)
    dpad = np.zeros((B, 256, H), dtype=np.float16)
    dpad[:, :H, :] = d
    dflat = dpad.reshape(B, 256 * H)
    w = np.asarray(weight).reshape(OC, KS, KS).astype(np.float16)

    wbd = np.zeros((2, KPMAX, 3 * OC), dtype=np.float16)
    wbd[:, :, 0:OC] = w.reshape(OC, KS * KS).T
    for m in range(2):
        for ky in range(KS):
            for kxl in range(4):
                kx = m * 4 + kxl
                if kx >= KS:
                    continue
                wbd[:, ky * 4 + kxl, OC * (1 + m):OC * (2 + m)] = w[:, ky, kx]

    in_maps = []
    for c in range(NCORES):
        i2cd = np.zeros((KPMAX, NTILES, RUN), dtype=np.float16)
        for t in range(NTILES):
            imgi, blk = divmod(t, NBLK)
            g = c * IPC + imgi
            r0 = BLK * blk
            kxl = PAIR_KXL[t // 2]
            for ky in range(KS):
                base = (r0 + ky) * H
                for kxi in range(kxl):
                    i2cd[ky * kxl + kxi, t, :] = \
                        dflat[g, base + kxi:base + kxi + RUN]
        in_maps.append({"i2cd": i2cd, "wbd": wbd})
    return in_maps


def kernel(data, weight):
    from concourse.bass_utils import run_bass_kernel_spmd

    if "nc" not in _CACHE:
        _CACHE["nc"] = _build()
    nc = _CACHE["nc"]

    in_maps = _prep_inputs(np.asarray(data), np.asarray(weight))
    res = run_bass_kernel_spmd(nc, in_maps, core_ids=list(range(NCORES)))
    outs = [r["out"] for r in res.results]
    full = np.concatenate(outs, axis=0)  # [32, 64, 218, 218] f16
    return full.astype(np.float32)
